# revision 1
# baseline (speedup 1.0000x reference)
"""DGCNN (nn_DGCNN_32727650795899) Trainium2 Bass kernel.

Sharding: B=4 samples x 2 row-halves -> 8 cores. Core c handles sample c//2,
point rows [ (c%2)*2048, (c%2+1)*2048 ). Weights replicated. Pairs of cores
exchange x1/x2 feature halves via AllGather and the global-max vector via
AllReduce(max).

Self-contained: hardcodes all shapes; builds/compiles the Bass program on
first call and runs it on NeuronCores 0-7 via run_bass_kernel_spmd.
"""

import contextlib
import sys

sys.path.insert(0, "/opt/trn_rl_repo")

import numpy as np

from concourse import bacc, mybir, tile

FP32 = mybir.dt.float32
U16 = mybir.dt.uint16
I16 = mybir.dt.int16
ACT = mybir.ActivationFunctionType
ALU = mybir.AluOpType

B = 4
C0 = 9
N = 4096
HALF = N // 2          # rows per core
NBLK = HALF // 128     # 16 row blocks per core
KNN = 20
EPS = 1e-5
NEG = -3.0e38

# packed bias tile columns: (offset, width)
BIAS_LAYOUT = {
    "b1_02": (0, 1), "b1_08": (1, 1), "b2_02": (2, 1), "b2_08": (3, 1),
    "b3_02": (4, 1), "b3_08": (5, 1), "b4_02": (6, 1), "b4_08": (7, 1),
    "b5_02": (8, 1), "b5_08": (9, 1),
    "b6_02": (10, 8), "b6_08": (18, 8),
    "b7_02": (26, 4), "b7_08": (30, 4),
    "b8_02": (34, 2), "b8_08": (36, 2),
}
BIAS_W = 38
# packed small-weights tile columns (each 64 wide): order
WPACK = ["wnT1", "bwT1", "w2T", "wnT3", "bwT3", "w4T", "wnT5", "bwT5"]

_CACHE = {}
ABLATE = set()  # timing-sim ablations: 'topk','gather','ttadd','redk','pdmm','pdcopy'


def _edge_layer(nc, tc, pools, Cin, xmy_aug, xfull, wn, bw, b02, b08,
                w2T, cb02, cb08, out_rows, lname):
    """One EdgeConv block over this core's 2048 rows (see module docstring)."""
    pool, ppool, zpool, ones64 = pools
    lctx = contextlib.ExitStack()
    lpool = lctx.enter_context(tc.tile_pool(name=f"L{lname}", bufs=1))

    # ---- per-layer precompute over the full 4096 columns ----
    x2s_aug = lpool.tile([Cin + 1, N], FP32, name=f"x2s_{lname}")
    nc.scalar.activation(out=x2s_aug[0:Cin, :], in_=xfull, func=ACT.Copy,
                         scale=2.0)
    for ch in range(8):
        sl = slice(ch * 512, (ch + 1) * 512)
        xsq = pool.tile([Cin, 512], FP32, tag="xsq", bufs=2,
                        name=f"xsq_{lname}_{ch}")
        nc.scalar.activation(out=xsq[:], in_=xfull[:, sl], func=ACT.Square)
        pp = ppool.tile([128, 512], FP32, tag="mm", name=f"ppxx_{lname}_{ch}")
        nc.tensor.matmul(pp[0:1, :], ones64[0:Cin, 0:1], xsq[:], start=True,
                         stop=True)
        xxn = pool.tile([1, 512], FP32, tag="xxn", bufs=2,
                        name=f"xxn_{lname}_{ch}")
        nc.scalar.activation(out=xxn[:], in_=pp[0:1, :], func=ACT.Copy,
                             scale=-1.0)
        # row Cin of x2s_aug = -xx (DMA; compute engines can't start there)
        nc.sync.dma_start(x2s_aug[Cin:Cin + 1, sl], xxn[:])

    # A = Wn^T-fold @ xfull [64, 4096]; Bt = Bw @ x_my [64, 2048]
    A = lpool.tile([64, N], FP32, name=f"A_{lname}")
    for ch in range(8):
        sl = slice(ch * 512, (ch + 1) * 512)
        pp = ppool.tile([128, 512], FP32, tag="mm", name=f"ppA_{lname}_{ch}")
        nc.tensor.matmul(pp[0:64, :], wn, xfull[:, sl], start=True, stop=True)
        nc.scalar.activation(out=A[:, sl], in_=pp[0:64, :], func=ACT.Copy)
    Bt = lpool.tile([64, HALF], FP32, name=f"B_{lname}")
    for ch in range(4):
        sl = slice(ch * 512, (ch + 1) * 512)
        pp = ppool.tile([128, 512], FP32, tag="mm", name=f"ppB_{lname}_{ch}")
        nc.tensor.matmul(pp[0:64, :], bw, xmy_aug[0:Cin, sl], start=True,
                         stop=True)
        nc.scalar.activation(out=Bt[:, sl], in_=pp[0:64, :], func=ACT.Copy)

    # ---- per 128-row block ----
    for blk in range(NBLK):
        rsl = slice(blk * 128, (blk + 1) * 128)
        pd = pool.tile([128, N], FP32, tag="pd", bufs=2, name=f"pd_{lname}_{blk}")
        for ch in range(8):
            sl = slice(ch * 512, (ch + 1) * 512)
            pp = ppool.tile([128, 512], FP32, tag="mm",
                            name=f"ppd_{lname}_{blk}_{ch}")
            if "pdmm" not in ABLATE:
                nc.tensor.matmul(pp[:], xmy_aug[:, rsl], x2s_aug[:, sl],
                                 start=True, stop=True)
            if "pdcopy" not in ABLATE:
                nc.scalar.activation(out=pd[:, sl], in_=pp[:], func=ACT.Copy)
        # top-20 (exact; ties -> lowest index, like jax.lax.top_k)
        scr = pool.tile([128, 392], U16, tag="scr16", bufs=2,
                        name=f"scr_{lname}_{blk}")
        kidx = scr[:, 0:24]
        kidxw = scr[0:16, 24:184]
        idxr = scr[0:64, 184:344]
        m8 = pool.tile([128, 8], FP32, tag="m8", bufs=2, name=f"m8_{lname}_{blk}")
        if "topk" not in ABLATE:
            nc.vector.max(m8[:], pd[:])
            nc.vector.max_index(kidx[:, 0:8], m8[:], pd[:])
            nc.vector.match_replace(pd[:], m8[:], pd[:], NEG)
            nc.vector.max(m8[:], pd[:])
            nc.vector.max_index(kidx[:, 8:16], m8[:], pd[:])
            nc.vector.match_replace(pd[:], m8[:], pd[:], NEG)
            nc.vector.max(m8[:], pd[:])
            nc.vector.max_index(kidx[:, 16:24], m8[:], pd[:])
        else:
            nc.vector.memset(kidx[:], 0)
        # wrap + replicate indices for the 64-channel gather
        qengs = [nc.sync, nc.scalar, nc.sync, nc.scalar]
        for g in range(8):
            qengs[g % 4].dma_start(kidxw[:, g * 20:(g + 1) * 20],
                                   kidx[16 * g:16 * (g + 1), 0:20])
        for a in range(4):
            qengs[a].dma_start(idxr[16 * a:16 * (a + 1), :], kidxw)
        # gather neighbor features: E[c, g, k, r] = A[c, idx[16g+r, k]]
        E = pool.tile([64, 128 * KNN], FP32, tag="E", bufs=2,
                      name=f"E_{lname}_{blk}")
        if "gather" not in ABLATE:
            nc.gpsimd.ap_gather(
                E[:].unsqueeze(-1), A[:].unsqueeze(-1), idxr[:].bitcast(I16),
                channels=64, num_elems=N, d=1, num_idxs=128 * KNN)
        else:
            nc.vector.memset(E[:], 0.0)
        # E += B_i (broadcast over k)
        ev = E[:].rearrange("c (g k r) -> c g k r", g=8, k=KNN)
        bv = Bt[:, rsl].rearrange("c (g r) -> c g r", g=8).unsqueeze(2) \
            .broadcast_to([64, 8, KNN, 16])
        if w2T is not None and "ttadd" not in ABLATE:
            nc.vector.tensor_tensor(out=ev, in0=ev, in1=bv, op=ALU.add)
        if w2T is not None:
            # lrelu via two rhs: W2@(0.2 y) + W2@relu(0.8 y), y = E + b
            r1 = pool.tile([64, 128 * KNN], FP32, tag="r1", bufs=1,
                           name=f"r1_{lname}_{blk}")
            r2 = pool.tile([64, 128 * KNN], FP32, tag="r2", bufs=1,
                           name=f"r2_{lname}_{blk}")
            nc.scalar.activation(out=r1[:], in_=E[:], func=ACT.Identity,
                                 scale=0.2, bias=b02)
            nc.scalar.activation(out=r2[:], in_=E[:], func=ACT.Relu,
                                 scale=0.8, bias=b08)
            z2 = zpool.tile([64, 128 * KNN], FP32, tag="z2",
                            name=f"z2_{lname}_{blk}")
            for ch in range(5):
                sl = slice(ch * 512, (ch + 1) * 512)
                nc.tensor.matmul(z2[:, sl], w2T, r1[:, sl], start=True,
                                 stop=False)
                nc.tensor.matmul(z2[:, sl], w2T, r2[:, sl], start=False,
                                 stop=True)
            red_src = z2
        else:
            red_src = E
        # max over k (lrelu is monotone -> activation after the reduce)
        yt = pool.tile([64, 3 * 128], FP32, tag="yt", bufs=2,
                       name=f"yt_{lname}_{blk}")
        y = yt[:, 0:128]
        t1 = yt[:, 128:256]
        t2 = yt[:, 256:384]
        if "redk" not in ABLATE:
            nc.vector.tensor_reduce(
                out=y.rearrange("c (g r) -> c g r", g=8),
                in_=red_src[:].rearrange("c (g k r) -> c g r k", g=8, k=KNN),
                axis=mybir.AxisListType.X, op=ALU.max)
        else:
            nc.vector.memset(y, 0.0)
        if w2T is None:
            # max_k (A_j + B_i) = (max_k A_j) + B_i
            nc.vector.tensor_tensor(out=y, in0=y, in1=Bt[:, rsl], op=ALU.add)
        nc.scalar.activation(out=t1, in_=y, func=ACT.Identity, scale=0.2,
                             bias=cb02)
        nc.scalar.activation(out=t2, in_=y, func=ACT.Relu, scale=0.8,
                             bias=cb08)
        nc.vector.tensor_tensor(out=out_rows[:, rsl], in0=t1, in1=t2,
                                op=ALU.add)
    lctx.close()


def build(pairs, reps=1):
    """Build + compile the SPMD program. pairs: replica groups (list of lists).
    reps: run the whole pipeline this many times (for slope-based timing)."""
    nc = bacc.Bacc("TRN2", target_bir_lowering=False, debug=False)

    def din(name, shape, dtype=FP32):
        return nc.dram_tensor(name, shape, dtype, kind="ExternalInput")

    X = din("x_full", [C0, N])
    XMY = din("xmy_aug", [C0 + 1, HALF])
    WS = din("wsmall", [64, 64 * len(WPACK)])
    BIASES = din("biases", [128, BIAS_W])
    W6T3 = din("w6T3", [64, 3072])
    W7XT3 = din("w7xT3", [64, 1536])
    W7GT8 = din("w7gT8", [128, 4096])
    W8T4 = din("w8T4", [128, 1024])
    W9T2 = din("w9T2", [128, 16])
    OUT = nc.dram_tensor("out", [8, HALF], FP32, kind="ExternalOutput")

    with tile.TileContext(nc) as tc:
        ctx = contextlib.ExitStack()
        persist = ctx.enter_context(tc.tile_pool(name="persist", bufs=1))
        ppool = ctx.enter_context(tc.tile_pool(name="ps", bufs=3, space="PSUM"))
        dpool = ctx.enter_context(tc.tile_pool(name="dram", bufs=1, space="DRAM"))

        ones64 = persist.tile([64, 1], FP32, name="ones64")
        nc.vector.memset(ones64[:], 1.0)

        wsmall = persist.tile([64, 64 * len(WPACK)], FP32, name="wsmall")
        nc.sync.dma_start(wsmall[:], WS[:])

        def wsl(name, Cin):
            j = WPACK.index(name)
            return wsmall[0:Cin, j * 64:(j + 1) * 64]

        biases = persist.tile([128, BIAS_W], FP32, name="biases")
        nc.sync.dma_start(biases[:], BIASES[:])

        def bsl(name, p=64):
            o, w = BIAS_LAYOUT[name]
            return biases[0:p, o:o + w]

        # feature tiles; x2my[0:10] doubles as layer-1 x_my storage,
        # xf (64x4096) holds x (rows 0:9) then x1full then x2full.
        x1my = persist.tile([65, HALF], FP32, name="x1my")
        x2my = persist.tile([65, HALF], FP32, name="x2my")
        x3my = persist.tile([64, HALF], FP32, name="x3my")
        xf = persist.tile([64, N], FP32, name="xf")

        for _rep in range(reps):
            nc.sync.dma_start(xf[0:C0, :], X[:])
            nc.sync.dma_start(x2my[0:C0 + 1, :], XMY[:])
            nc.sync.dma_start(x1my[64:65, :], XMY[C0:C0 + 1, :])
            ectx = contextlib.ExitStack()
            pool = ectx.enter_context(tc.tile_pool(name="work", bufs=1))
            zpool = ectx.enter_context(tc.tile_pool(name="psz", bufs=1, space="PSUM"))
            pools = (pool, ppool, zpool, ones64)

            def ag_half(half_ap, full_ap, name):
                if pairs is None:  # timing-sim variant: fake the exchange locally
                    bi = dpool.tile([64, HALF], FP32, name=f"agi_{name}")
                    nc.sync.dma_start(bi[:], half_ap)
                    nc.sync.dma_start(full_ap[:, 0:HALF], bi[:])
                    nc.sync.dma_start(full_ap[:, HALF:N], bi[:])
                    return
                bi = dpool.tile([64, HALF], FP32, name=f"agi_{name}")
                bo = dpool.tile([2, 64, HALF], FP32, name=f"ago_{name}")
                nc.sync.dma_start(bi[:], half_ap)
                nc.gpsimd.collective_compute("AllGather", ALU.bypass,
                                             replica_groups=pairs,
                                             ins=[bi[:]], outs=[bo[:]])
                nc.sync.dma_start(full_ap[:, 0:HALF], bo[0])
                nc.sync.dma_start(full_ap[:, HALF:N], bo[1])

            # ---- layer 1 ----
            _edge_layer(nc, tc, pools, C0, x2my[0:C0 + 1, :], xf[0:C0, :],
                        wsl("wnT1", C0), wsl("bwT1", C0),
                        bsl("b1_02"), bsl("b1_08"), wsl("w2T", 64),
                        bsl("b2_02"), bsl("b2_08"), x1my[0:64, :], "l1")
            ag_half(x1my[0:64, :], xf, "x1")

            # ---- layer 2 ---- (x2my rows get overwritten only after layer-1 reads)
            _edge_layer(nc, tc, pools, 64, x1my, xf, wsl("wnT3", 64),
                        wsl("bwT3", 64), bsl("b3_02"), bsl("b3_08"), wsl("w4T", 64),
                        bsl("b4_02"), bsl("b4_08"), x2my[0:64, :], "l2")
            nc.sync.dma_start(x2my[64:65, :], XMY[C0:C0 + 1, :])
            ag_half(x2my[0:64, :], xf, "x2")

            # ---- layer 3 ----
            _edge_layer(nc, tc, pools, 64, x2my, xf, wsl("wnT5", 64),
                        wsl("bwT5", 64), None, None, None,
                        bsl("b5_02"), bsl("b5_08"), x3my, "l3")
            ectx.close()

            # ---- head ----
            hctx = contextlib.ExitStack()
            hpool = hctx.enter_context(tc.tile_pool(name="head", bufs=1))
            w6T3 = hpool.tile([64, 3072], FP32, name="w6t")
            nc.sync.dma_start(w6T3[:], W6T3[:])
            w7xT3 = hpool.tile([64, 1536], FP32, name="w7xt")
            nc.sync.dma_start(w7xT3[:], W7XT3[:])
            w7gT8 = hpool.tile([128, 4096], FP32, name="w7gt")
            nc.sync.dma_start(w7gT8[:], W7GT8[:])
            w8T4 = hpool.tile([128, 1024], FP32, name="w8t")
            nc.sync.dma_start(w8T4[:], W8T4[:])
            w9T2 = hpool.tile([128, 16], FP32, name="w9t")
            nc.sync.dma_start(w9T2[:], W9T2[:])

            cats = [x1my, x2my, x3my]  # rows 0:64 each

            # y6max[p, m] = max_n (W6 @ cat)[m*128+p, n]
            y6max = hpool.tile([128, 8], FP32, name="y6max")
            for m in range(8):
                y6p = hpool.tile([128, 4], FP32, tag="y6p", bufs=2, name=f"y6p_{m}")
                for nch in range(4):
                    sl = slice(nch * 512, (nch + 1) * 512)
                    pp = ppool.tile([128, 512], FP32, tag="mm", name=f"z6_{m}_{nch}")
                    for j in range(3):
                        nc.tensor.matmul(pp[:], w6T3[:, j * 1024 + m * 128:
                                                     j * 1024 + (m + 1) * 128],
                                         cats[j][0:64, sl], start=(j == 0),
                                         stop=(j == 2))
                    nc.vector.tensor_reduce(out=y6p[:, nch:nch + 1],
                                            in_=pp[:], axis=mybir.AxisListType.X,
                                            op=ALU.max)
                nc.vector.tensor_reduce(out=y6max[:, m:m + 1],
                                        in_=y6p[:], axis=mybir.AxisListType.X,
                                        op=ALU.max)
            # pair AllReduce(max), then leaky-relu
            gb_i = dpool.tile([128, 8], FP32, name="ar_i")
            gb_o = dpool.tile([128, 8], FP32, name="ar_o")
            nc.sync.dma_start(gb_i[:], y6max[:])
            if pairs is not None:
                nc.gpsimd.collective_compute("AllReduce", ALU.max,
                                             replica_groups=pairs,
                                             ins=[gb_i[:]], outs=[gb_o[:]])
            else:
                nc.sync.dma_start(gb_o[:], gb_i[:])
            gmxpre = hpool.tile([128, 8], FP32, name="gmxpre")
            nc.sync.dma_start(gmxpre[:], gb_o[:])
            gmx = hpool.tile([128, 8], FP32, name="gmx")
            g1 = hpool.tile([128, 8], FP32, name="g1t")
            g2 = hpool.tile([128, 8], FP32, name="g2t")
            for m in range(8):
                nc.scalar.activation(out=g1[:, m:m + 1], in_=gmxpre[:, m:m + 1],
                                     func=ACT.Identity, scale=0.2,
                                     bias=bsl("b6_02", 128)[:, m:m + 1])
                nc.scalar.activation(out=g2[:, m:m + 1], in_=gmxpre[:, m:m + 1],
                                     func=ACT.Relu, scale=0.8,
                                     bias=bsl("b6_08", 128)[:, m:m + 1])
            nc.vector.tensor_tensor(out=gmx[:], in0=g1[:], in1=g2[:], op=ALU.add)

            # b7eff = W7g @ gmx + b7 (kept pre-scaled by 0.2 / 0.8)
            b7e2 = hpool.tile([128, 4], FP32, name="b7e2")
            b7e8 = hpool.tile([128, 4], FP32, name="b7e8")
            for m in range(4):
                pw = ppool.tile([128, 512], FP32, tag="mm", name=f"w7g_{m}")
                for k in range(8):
                    nc.tensor.matmul(pw[:, 0:1],
                                     w7gT8[:, k * 512 + m * 128:k * 512 + (m + 1) * 128],
                                     gmx[:, k:k + 1], start=(k == 0), stop=(k == 7))
                nc.scalar.activation(out=b7e2[:, m:m + 1], in_=pw[:, 0:1],
                                     func=ACT.Identity, scale=0.2,
                                     bias=bsl("b7_02", 128)[:, m:m + 1])
                nc.scalar.activation(out=b7e8[:, m:m + 1], in_=pw[:, 0:1],
                                     func=ACT.Identity, scale=0.8,
                                     bias=bsl("b7_08", 128)[:, m:m + 1])

            # h7 kept as 0.2y / relu(0.8y) parts (absorbed by the next matmul)
            h7a = hpool.tile([128, 8192], FP32, name="h7a")
            h7b = hpool.tile([128, 8192], FP32, name="h7b")
            for m in range(4):
                for nch in range(4):
                    sl = slice(nch * 512, (nch + 1) * 512)
                    osl = slice(m * 2048 + nch * 512, m * 2048 + (nch + 1) * 512)
                    pp = ppool.tile([128, 512], FP32, tag="mm", name=f"z7_{m}_{nch}")
                    for j in range(3):
                        nc.tensor.matmul(pp[:], w7xT3[:, j * 512 + m * 128:
                                                      j * 512 + (m + 1) * 128],
                                         cats[j][0:64, sl], start=(j == 0),
                                         stop=(j == 2))
                    nc.scalar.activation(out=h7a[:, osl], in_=pp[:],
                                         func=ACT.Identity, scale=0.2,
                                         bias=b7e2[:, m:m + 1])
                    nc.scalar.activation(out=h7b[:, osl], in_=pp[:], func=ACT.Relu,
                                         scale=0.8, bias=b7e8[:, m:m + 1])

            # h8
            h8a = hpool.tile([128, 4096], FP32, name="h8a")
            h8b = hpool.tile([128, 4096], FP32, name="h8b")
            for m8 in range(2):
                for nch in range(4):
                    osl = slice(m8 * 2048 + nch * 512, m8 * 2048 + (nch + 1) * 512)
                    pp = ppool.tile([128, 512], FP32, tag="mm", name=f"z8_{m8}_{nch}")
                    first = True
                    for k in range(4):
                        ksl = slice(k * 2048 + nch * 512, k * 2048 + (nch + 1) * 512)
                        lhs = w8T4[:, k * 256 + m8 * 128:k * 256 + (m8 + 1) * 128]
                        nc.tensor.matmul(pp[:], lhs, h7a[:, ksl], start=first,
                                         stop=False)
                        first = False
                        nc.tensor.matmul(pp[:], lhs, h7b[:, ksl], start=False,
                                         stop=(k == 3))
                    nc.scalar.activation(out=h8a[:, osl], in_=pp[:],
                                         func=ACT.Identity, scale=0.2,
                                         bias=bsl("b8_02", 128)[:, m8:m8 + 1])
                    nc.scalar.activation(out=h8b[:, osl], in_=pp[:], func=ACT.Relu,
                                         scale=0.8,
                                         bias=bsl("b8_08", 128)[:, m8:m8 + 1])

            # out = W9 @ h8
            outsb = hpool.tile([8, HALF], FP32, name="outsb")
            for nch in range(4):
                sl = slice(nch * 512, (nch + 1) * 512)
                pp = ppool.tile([128, 512], FP32, tag="mm", name=f"z9_{nch}")
                first = True
                for k2 in range(2):
                    ksl = slice(k2 * 2048 + nch * 512, k2 * 2048 + (nch + 1) * 512)
                    lhs = w9T2[:, k2 * 8:(k2 + 1) * 8]
                    nc.tensor.matmul(pp[0:8, :], lhs, h8a[:, ksl], start=first,
                                     stop=False)
                    first = False
                    nc.tensor.matmul(pp[0:8, :], lhs, h8b[:, ksl], start=False,
                                     stop=(k2 == 1))
                nc.scalar.activation(out=outsb[:, sl], in_=pp[0:8, :], func=ACT.Copy)
            nc.sync.dma_start(OUT[:], outsb[:])
            hctx.close()
        ctx.close()

    nc.compile()
    return nc


def _prep_in_maps(x, W1, W2, W3, W4, W5, W6, W7, W8, W9,
                  g1, b1, g2, b2, g3, b3, g4, b4, g5, b5, g6, b6, g7, b7, g8, b8):
    f = np.float32
    sc = {i: (g / np.sqrt(f(1.0) + f(EPS))).astype(f) for i, g in
          [(1, g1), (2, g2), (3, g3), (4, g4), (5, g5), (6, g6), (7, g7), (8, g8)]}

    def fold(W, s):
        return (W * s[:, None]).astype(f)

    W1f = fold(W1, sc[1]); W2f = fold(W2, sc[2]); W3f = fold(W3, sc[3])
    W4f = fold(W4, sc[4]); W5f = fold(W5, sc[5]); W6f = fold(W6, sc[6])
    W7f = fold(W7, sc[7]); W8f = fold(W8, sc[8])

    def edge_w(Wf, Cin):
        wn = Wf[:, :Cin]
        bw = Wf[:, Cin:] - wn
        return np.ascontiguousarray(wn.T), np.ascontiguousarray(bw.T)

    wnT1, bwT1 = edge_w(W1f, C0)
    wnT3, bwT3 = edge_w(W3f, 64)
    wnT5, bwT5 = edge_w(W5f, 64)

    wsmall = np.zeros((64, 64 * len(WPACK)), f)
    parts = dict(wnT1=wnT1, bwT1=bwT1, w2T=W2f.T, wnT3=wnT3, bwT3=bwT3,
                 w4T=W4f.T, wnT5=wnT5, bwT5=bwT5)
    for j, nm in enumerate(WPACK):
        p = parts[nm]
        wsmall[0:p.shape[0], j * 64:j * 64 + p.shape[1]] = p

    biases = np.zeros((128, BIAS_W), f)
    for nm, bvec in [("b1", b1), ("b2", b2), ("b3", b3), ("b4", b4), ("b5", b5),
                     ("b6", b6), ("b7", b7), ("b8", b8)]:
        for suf, s in [("_02", f(0.2)), ("_08", f(0.8))]:
            o, w = BIAS_LAYOUT[nm + suf]
            bm = (s * bvec.astype(f)).reshape(w, -1).T  # [p, w]
            biases[0:bm.shape[0], o:o + w] = bm

    W6T = W6f.T
    w6T3 = np.concatenate([W6T[0:64], W6T[64:128], W6T[128:192]], axis=1)
    W7g = W7f[:, :1024]; W7x = W7f[:, 1024:]
    W7xT = W7x.T
    w7xT3 = np.concatenate([W7xT[0:64], W7xT[64:128], W7xT[128:192]], axis=1)
    W7gT = W7g.T
    w7gT8 = np.concatenate([W7gT[k * 128:(k + 1) * 128] for k in range(8)], axis=1)
    W8T = W8f.T
    w8T4 = np.concatenate([W8T[k * 128:(k + 1) * 128] for k in range(4)], axis=1)
    W9T = W9.astype(f).T
    w9T2 = np.concatenate([W9T[0:128], W9T[128:256]], axis=1)

    com = dict(wsmall=wsmall, biases=biases,
               w6T3=np.ascontiguousarray(w6T3),
               w7xT3=np.ascontiguousarray(w7xT3),
               w7gT8=np.ascontiguousarray(w7gT8),
               w8T4=np.ascontiguousarray(w8T4),
               w9T2=np.ascontiguousarray(w9T2))

    in_maps = []
    for c in range(2 * B):
        s, h = c // 2, c % 2
        xs = np.asarray(x[s], dtype=f)
        xmy = xs[:, h * HALF:(h + 1) * HALF]
        xmy_aug = np.concatenate([xmy, np.ones((1, HALF), f)], axis=0)
        m = dict(com)
        m["x_full"] = np.ascontiguousarray(xs)
        m["xmy_aug"] = np.ascontiguousarray(xmy_aug)
        in_maps.append(m)
    return in_maps


def _build_executor(nc, n_cores):
    """Cached jitted PJRT executor (run_bass_kernel_spmd re-lowers per call)."""
    import jax
    from jax.sharding import Mesh, PartitionSpec
    from jax.experimental.shard_map import shard_map
    from concourse.bass2jax import (
        install_neuronx_cc_hook, _bass_exec_p, partition_id_tensor)

    install_neuronx_cc_hook()
    partition_name = (nc.partition_id_tensor.name
                      if nc.partition_id_tensor else None)
    in_names, out_names, out_avals, zero_shapes = [], [], [], []
    for alloc in nc.m.functions[0].allocations:
        if not isinstance(alloc, mybir.MemoryLocationSet):
            continue
        name = alloc.memorylocations[0].name
        if alloc.kind == "ExternalInput":
            if name != partition_name:
                in_names.append(name)
        elif alloc.kind == "ExternalOutput":
            shape = tuple(alloc.tensor_shape)
            dtype = mybir.dt.np(alloc.dtype)
            out_names.append(name)
            out_avals.append(jax.core.ShapedArray(shape, dtype))
            zero_shapes.append((shape, dtype))
    n_params = len(in_names)
    n_outs = len(out_avals)
    all_names = in_names + out_names
    if partition_name is not None:
        all_names.append(partition_name)

    def _body(*args):
        operands = list(args)
        if partition_name is not None:
            operands.append(partition_id_tensor())
        return tuple(_bass_exec_p.bind(
            *operands, out_avals=tuple(out_avals), in_names=tuple(all_names),
            out_names=tuple(out_names), lowering_input_output_aliases=(),
            sim_require_finite=True, sim_require_nnan=True, nc=nc))

    devices = jax.devices()[:n_cores]
    mesh = Mesh(np.asarray(devices), ("core",))
    in_specs = (PartitionSpec("core"),) * (n_params + n_outs)
    out_specs = (PartitionSpec("core"),) * n_outs
    donate = tuple(range(n_params, n_params + n_outs))
    fn = jax.jit(shard_map(_body, mesh=mesh, in_specs=in_specs,
                           out_specs=out_specs, check_rep=False),
                 donate_argnums=donate, keep_unused=True)

    def run(in_maps):
        concat_in = [np.concatenate([np.asarray(in_maps[c][nm])
                                     for c in range(n_cores)], axis=0)
                     for nm in in_names]
        zeros = [np.zeros((n_cores * s[0], *s[1:]), d) for s, d in zero_shapes]
        outs = fn(*concat_in, *zeros)
        return [{nm: np.asarray(outs[i]).reshape(n_cores, *out_avals[i].shape)[c]
                 for i, nm in enumerate(out_names)} for c in range(n_cores)]

    return run


def kernel(**inputs):
    inputs = {k: np.asarray(v, dtype=np.float32) for k, v in inputs.items()}
    if "nc" not in _CACHE:
        _CACHE["nc"] = build([[0, 1], [2, 3], [4, 5], [6, 7]])
        _CACHE["run"] = _build_executor(_CACHE["nc"], 2 * B)
    in_maps = _prep_in_maps(**inputs)
    results = _CACHE["run"](in_maps)
    out = np.empty((B, 8, N), dtype=np.float32)
    for c in range(2 * B):
        s, h = c // 2, c % 2
        out[s, :, h * HALF:(h + 1) * HALF] = results[c]["out"]
    return out

